# revision 18
# baseline (speedup 1.0000x reference)
"""Fused GNN edge-MLP kernel for Trainium2 (8 NeuronCores, batch-parallel).

Model (per batch b of 16; NN=256 nodes, FD=128):
  feat = [node_type(8) | pos-embedding(60) | quat(4)]            (256, 72)
  x    = relu(relu(feat @ W1 + b1) @ W2 + b2)                    (256, 128)
  e_ij = x_i + x_j   for all i<j pairs (NE = 32640)
  out  = (relu(relu(e @ We1 + be1) @ We2 + be2)) @ Wo + bo       (NB, NE)

Key algebraic restructure: relu(e_ij@We1 + be1) = relu(y_i + y_j) with
y = x @ We1 + be1/2 — the first edge GEMM collapses to one 256-col GEMM
per batch.  Each core handles 2 batches fully on-chip; no collectives.

Edge columns are generated anchor-major with per-anchor widths padded to
even (total exactly 32768 = 64 x 512 tiles); the host drops pad columns
and reorders to triu_indices order afterwards.
"""

import math
import numpy as np

import concourse.bass as bass
import concourse.mybir as mybir
from concourse import bacc
from concourse.tile import TileContext
from concourse.bass_utils import run_bass_kernel_spmd

F32 = mybir.dt.float32
FP16 = mybir.dt.float16
ALU = mybir.AluOpType
ACTF = mybir.ActivationFunctionType

NB, NN, FD = 16, 256, 128
POS_D, TYPE_D, QUAT_D = 3, 8, 4
PE_OCT = 10
DIN = 72
NCORES = 8
BPC = NB // NCORES          # batches per core = 2
NE = NN * (NN - 1) // 2     # 32640
CPAD = 32768                # padded gen-order edge columns per batch
TILE = 512
NTILES = CPAD // TILE       # 64
YSTRIDE = 262               # y columns per batch inside the y tile (256 + 6 pad)

PI = float(np.float32(np.pi))


# --------------------------------------------------------------------------
# static edge-run bookkeeping (python-level, baked into the kernel)
# --------------------------------------------------------------------------
def _build_runs():
    """Per 512-tile: list of (lo, ln, i, k0) pieces.

    Gen-order column  B(i)+k  holds edge (i, i+1+k), k < padded width wp(i).
    """
    runs_by_tile = [[] for _ in range(NTILES)]
    pos = 0
    for i in range(NN - 1):
        w = NN - 1 - i
        wp = w + (w & 1)
        start = pos
        while start < pos + wp:
            t = start // TILE
            end = min(pos + wp, (t + 1) * TILE)
            runs_by_tile[t].append((start - t * TILE, end - start, i, start - pos))
            start = end
        pos += wp
    assert pos == CPAD
    return runs_by_tile


def _gather_index():
    """lex edge index -> gen-order position."""
    idx = np.empty(NE, np.int64)
    pos = 0
    e = 0
    for i in range(NN - 1):
        w = NN - 1 - i
        wp = w + (w & 1)
        idx[e:e + w] = np.arange(pos, pos + w)
        e += w
        pos += wp
    return idx


RUNS_BY_TILE = _build_runs()
GATHER = _gather_index()

# Device feat row layout (32-aligned blocks for the engine partition-base rule):
#   rows  0:30  cos block (octave-major, xyz)
#   rows 30:32  zero pad
#   rows 32:62  sin block
#   rows 62:64  zero pad
#   rows 64:72  node_type rows
#   rows 72:76  quat rows
# W1 rows are permuted/padded on the host to match (pad rows = 0).
#
# PE features use exact fp32 range reduction: with h = 2^(i-1)*x (exact) and
# f = h - round(h) via the 2^23 magic constant, cos(2^i*pi*x) = cos(2*pi*f)
# = sin(pi/2 - 2*pi*|f|) and sin(2^i*pi*x) = sin(2*pi*f), all Sin args in
# [-pi, pi] (the ACT table's accurate domain).
DIN_PAD = 76
_COS = [8 + 6 * i + a for i in range(PE_OCT) for a in range(3)]
_SIN = [8 + 6 * i + 3 + a for i in range(PE_OCT) for a in range(3)]
MAGIC = float(2 ** 23)


def _prep_w1(W1):
    W1p = np.zeros((DIN_PAD, FD), np.float32)
    W1p[0:30] = W1[_COS]
    W1p[32:62] = W1[_SIN]
    W1p[64:72] = W1[0:8]
    W1p[72:76] = W1[68:72]
    return W1p


# --------------------------------------------------------------------------
# device kernel
# --------------------------------------------------------------------------
def build_kernel(repeat=1):
    """repeat>1 re-emits the whole compute body N times (same outputs) —
    used only for wall-clock-delta timing in bench.py."""
    nc = bacc.Bacc("TRN2", target_bir_lowering=False, debug=False)

    ntqT = nc.dram_tensor("ntqT", [TYPE_D + QUAT_D, BPC * NN], F32, kind="ExternalInput")
    posT = nc.dram_tensor("posT", [POS_D, BPC * NN], F32, kind="ExternalInput")
    w1 = nc.dram_tensor("w1", [DIN_PAD, FD], F32, kind="ExternalInput")
    w2 = nc.dram_tensor("w2", [FD, FD], F32, kind="ExternalInput")
    we1 = nc.dram_tensor("we1", [FD, FD], F32, kind="ExternalInput")
    we2 = nc.dram_tensor("we2", [FD, FD], F32, kind="ExternalInput")
    wo = nc.dram_tensor("wo", [FD, 1], F32, kind="ExternalInput")
    b1 = nc.dram_tensor("b1", [FD, 1], F32, kind="ExternalInput")
    b2 = nc.dram_tensor("b2", [FD, 1], F32, kind="ExternalInput")
    be1h = nc.dram_tensor("be1h", [FD, 1], F32, kind="ExternalInput")
    be2 = nc.dram_tensor("be2", [FD, 1], F32, kind="ExternalInput")
    octs = nc.dram_tensor("octs", [3 * PE_OCT, 1], F32, kind="ExternalInput")
    out = nc.dram_tensor("out", [BPC, FD, CPAD // FD], F32, kind="ExternalOutput")

    NCOL = BPC * NN  # 512

    with TileContext(nc) as tc:
        with (
            tc.tile_pool(name="const", bufs=1) as cpool,
            tc.tile_pool(name="h1p", bufs=3) as h1pool,
            tc.tile_pool(name="h2p", bufs=3) as h2pool,
            tc.tile_pool(name="sp", bufs=2) as spool,
            tc.tile_pool(name="psmm", bufs=2, space="PSUM") as psmm,
            tc.tile_pool(name="pss", bufs=2, space="PSUM") as pss,
        ):
            # ---- load constants -------------------------------------------------
            w1_sb = cpool.tile([DIN_PAD, FD], F32, tag="w1")
            w2_sb = cpool.tile([FD, FD], F32, tag="w2")
            we1_sb = cpool.tile([FD, FD], F32, tag="we1")
            we2_sb = cpool.tile([FD, FD], F32, tag="we2")
            wo32_sb = cpool.tile([FD, 1], F32, tag="wo32")
            wo_sb = cpool.tile([FD, 1], FP16, tag="wo")
            b1_sb = cpool.tile([FD, 1], F32, tag="b1")
            b2_sb = cpool.tile([FD, 1], F32, tag="b2")
            be1h_sb = cpool.tile([FD, 1], F32, tag="be1h")
            be2_sb = cpool.tile([FD, 1], F32, tag="be2")
            octs_sb = cpool.tile([3 * PE_OCT, 1], F32, tag="octs")
            for sb, dr in [
                (w1_sb, w1), (w2_sb, w2), (we1_sb, we1), (we2_sb, we2),
                (wo32_sb, wo), (b1_sb, b1), (b2_sb, b2), (be1h_sb, be1h),
                (be2_sb, be2), (octs_sb, octs),
            ]:
                nc.sync.dma_start(sb[:], dr[:])
            nc.vector.tensor_copy(wo_sb[:], wo32_sb[:])  # cast fp32 -> bf16

            for _rep in range(repeat):
                _emit_body(nc, cpool, h1pool, h2pool, spool, psmm, pss,
                           w1_sb, w2_sb, we1_sb, we2_sb, wo_sb,
                           b1_sb, b2_sb, be1h_sb, be2_sb, octs_sb,
                           ntqT, posT, out)

    nc.compile()
    return nc


def _emit_body(nc, cpool, h1pool, h2pool, spool, psmm, pss,
               w1_sb, w2_sb, we1_sb, we2_sb, wo_sb,
               b1_sb, b2_sb, be1h_sb, be2_sb, octs_sb, ntqT, posT, out):
    NCOL = BPC * NN
    if True:
        if True:
            # ---- node stage (both batches share the 512-wide free dim) ----------
            feat = cpool.tile([FD, NCOL], F32, tag="feat")  # rows 0..71 used
            V = cpool.tile([3 * PE_OCT, NCOL], F32, tag="V")
            vt = cpool.tile([3 * PE_OCT, NCOL], F32, tag="vt")
            wt = cpool.tile([3 * PE_OCT, NCOL], F32, tag="wt")
            qt = cpool.tile([3 * PE_OCT, NCOL], F32, tag="qt")
            x1 = cpool.tile([FD, NCOL], F32, tag="x1")
            x2 = cpool.tile([FD, NCOL], F32, tag="x2")
            y = cpool.tile([FD, 2 * YSTRIDE], F32, tag="y")
            halfpi_sb = cpool.tile([3 * PE_OCT, 1], F32, tag="halfpi")
            nc.vector.memset(halfpi_sb[:], PI / 2)

            nc.vector.memset(feat[:], 0.0)
            nc.sync.dma_start(feat[64:76, :], ntqT[:])
            for k in range(PE_OCT):
                nc.sync.dma_start(V[3 * k:3 * k + 3, :], posT[:])
            # h = 2^(i-1)*x; r = round(h) via magic; f = h - r in [-0.5, 0.5]
            nc.vector.tensor_scalar(
                out=vt[:], in0=V[:], scalar1=octs_sb[:], scalar2=None, op0=ALU.mult)
            nc.vector.tensor_scalar(
                out=wt[:], in0=vt[:], scalar1=MAGIC, scalar2=MAGIC,
                op0=ALU.add, op1=ALU.subtract)
            nc.vector.tensor_sub(qt[:], vt[:], wt[:])          # f
            nc.scalar.activation(out=vt[:], in_=qt[:], func=ACTF.Abs,
                                 bias=0.0, scale=1.0)           # |f| (reuse vt)
            nc.scalar.activation(out=feat[0:30, :], in_=vt[:], func=ACTF.Sin,
                                 bias=halfpi_sb[:], scale=-2 * PI)
            nc.scalar.activation(out=feat[32:62, :], in_=qt[:], func=ACTF.Sin,
                                 bias=0.0, scale=2 * PI)

            ps1 = psmm.tile([FD, NCOL], F32, tag="mm")
            nc.tensor.matmul(out=ps1[:], lhsT=w1_sb[:], rhs=feat[0:DIN_PAD, :],
                             start=True, stop=True)
            nc.scalar.activation(out=x1[:], in_=ps1[:], func=ACTF.Relu, bias=b1_sb[:])
            ps2 = psmm.tile([FD, NCOL], F32, tag="mm")
            nc.tensor.matmul(out=ps2[:], lhsT=w2_sb[:], rhs=x1[:], start=True, stop=True)
            nc.scalar.activation(out=x2[:], in_=ps2[:], func=ACTF.Relu, bias=b2_sb[:])
            ps3 = psmm.tile([FD, NCOL], F32, tag="mm")
            nc.tensor.matmul(out=ps3[:], lhsT=we1_sb[:], rhs=x2[:], start=True, stop=True)
            nc.vector.memset(y[:], 0.0)
            for b in range(BPC):
                nc.scalar.activation(out=y[:, b * YSTRIDE:b * YSTRIDE + NN],
                                     in_=ps3[:, b * NN:(b + 1) * NN],
                                     func=ACTF.Identity, bias=be1h_sb[:])

            # ---- edge stage -----------------------------------------------------
            for b in range(BPC):
                yb = b * YSTRIDE
                s_sb = spool.tile([FD, CPAD // FD], F32, tag="s")
                for g in range(NTILES // 4):
                    ps_s = pss.tile([FD, 16], F32, tag="ps_s")
                    for tt in range(4):
                        t = 4 * g + tt
                        h1 = h1pool.tile([FD, TILE], F32, tag="h1")
                        for (lo, ln, i, k0) in RUNS_BY_TILE[t]:
                            nc.vector.tensor_scalar(
                                out=h1[:, lo:lo + ln],
                                in0=y[:, yb + i + 1 + k0: yb + i + 1 + k0 + ln],
                                scalar1=y[:, yb + i: yb + i + 1],
                                scalar2=0.0, op0=ALU.add, op1=ALU.max)
                        ps_e = psmm.tile([FD, TILE], F32, tag="mm")
                        nc.tensor.matmul(out=ps_e[:], lhsT=we2_sb[:], rhs=h1[:],
                                         start=True, stop=True)
                        h2 = h2pool.tile([FD, TILE], FP16, tag="h2")
                        nc.scalar.activation(out=h2[:], in_=ps_e[:], func=ACTF.Relu,
                                             bias=be2_sb[:])
                        for k in range(4):
                            nc.tensor.matmul(
                                out=ps_s[:, 4 * tt + k:4 * tt + k + 1],
                                lhsT=h2[:, FD * k:FD * (k + 1)],
                                rhs=wo_sb[:], start=True, stop=True)
                    nc.vector.tensor_copy(out=s_sb[:, 16 * g:16 * (g + 1)], in_=ps_s[:])
                nc.sync.dma_start(out[b], s_sb[:])


_NC_CACHE = None


def _get_nc():
    global _NC_CACHE
    if _NC_CACHE is None:
        _NC_CACHE = build_kernel()
    return _NC_CACHE


# --------------------------------------------------------------------------
# host wrapper
# --------------------------------------------------------------------------
def kernel(node_type, pos, quat, W1, b1, W2, b2, We1, be1, We2, be2, Wo, bo):
    node_type = np.asarray(node_type, np.float32)
    pos = np.asarray(pos, np.float32)
    quat = np.asarray(quat, np.float32)

    W1p = _prep_w1(np.asarray(W1, np.float32))

    octscale = np.repeat((2.0 ** (np.arange(PE_OCT) - 1)).astype(np.float32), 3)[:, None]
    col = lambda v: np.ascontiguousarray(np.asarray(v, np.float32).reshape(FD, 1))

    shared = {
        "w1": np.ascontiguousarray(W1p),
        "w2": np.ascontiguousarray(np.asarray(W2, np.float32)),
        "we1": np.ascontiguousarray(np.asarray(We1, np.float32)),
        "we2": np.ascontiguousarray(np.asarray(We2, np.float32)),
        "wo": np.ascontiguousarray(np.asarray(Wo, np.float32).reshape(FD, 1)),
        "b1": col(b1), "b2": col(b2),
        "be1h": col(np.asarray(be1, np.float32) * 0.5), "be2": col(be2),
        "octs": np.ascontiguousarray(octscale),
    }

    in_maps = []
    for c in range(NCORES):
        bs = slice(c * BPC, (c + 1) * BPC)
        nt = node_type[bs]          # [2, 256, 8]
        qt = quat[bs]               # [2, 256, 4]
        ps = pos[bs]                # [2, 256, 3]
        ntq = np.concatenate(
            [nt.transpose(2, 0, 1).reshape(TYPE_D, -1),
             qt.transpose(2, 0, 1).reshape(QUAT_D, -1)], axis=0)
        in_maps.append({
            "ntqT": np.ascontiguousarray(ntq),
            "posT": np.ascontiguousarray(ps.transpose(2, 0, 1).reshape(POS_D, -1)),
            **shared,
        })

    nc = _get_nc()
    results = run_bass_kernel_spmd(nc, in_maps, core_ids=list(range(NCORES))).results

    out = np.empty((NB, NE), np.float32)
    bo0 = np.float32(np.asarray(bo).reshape(-1)[0])
    for c in range(NCORES):
        r = results[c]["out"]  # [2, 128, 256]; element (p, col) = gen idx col*128+p
        for b in range(BPC):
            gen = np.ascontiguousarray(r[b].T).reshape(-1)
            out[c * BPC + b] = gen[GATHER] + bo0
    return out


# revision 45
# speedup vs baseline: 20.3417x; 20.3417x over previous
"""Fused GNN edge-MLP kernel for Trainium2 (8 NeuronCores, batch-parallel).

Model (per batch b of 16; NN=256 nodes, FD=128):
  feat = [node_type(8) | pos-embedding(60) | quat(4)]            (256, 72)
  x    = relu(relu(feat @ W1 + b1) @ W2 + b2)                    (256, 128)
  e_ij = x_i + x_j   for all i<j pairs (NE = 32640)
  out  = (relu(relu(e @ We1 + be1) @ We2 + be2)) @ Wo + bo       (NB, NE)

Key algebraic restructure: relu(e_ij@We1 + be1) = relu(y_i + y_j) with
y = x @ We1 + be1/2 — the first edge GEMM collapses to one 256-col GEMM
per batch.  Each core handles 2 batches fully on-chip; no collectives.

Edge columns are generated anchor-major with per-anchor widths padded to
even (total exactly 32768 = 64 x 512 tiles); the host drops pad columns
and reorders to triu_indices order afterwards.
"""

import math
import numpy as np

import concourse.bass as bass
import concourse.mybir as mybir
from concourse import bacc
from concourse.tile import TileContext
from concourse.bass_utils import run_bass_kernel_spmd

F32 = mybir.dt.float32
FP16 = mybir.dt.float16
ALU = mybir.AluOpType
ACTF = mybir.ActivationFunctionType

NB, NN, FD = 16, 256, 128
POS_D, TYPE_D, QUAT_D = 3, 8, 4
PE_OCT = 10
DIN = 72
NCORES = 8
BPC = NB // NCORES          # batches per core = 2
NE = NN * (NN - 1) // 2     # 32640
CPAD = 32768                # padded gen-order edge columns per batch
TILE = 512
NTILES = CPAD // TILE       # 64
YSTRIDE = 262               # y columns per batch inside the y tile (256 + 6 pad)

PI = float(np.float32(np.pi))


# --------------------------------------------------------------------------
# static edge-run bookkeeping (python-level, baked into the kernel)
# --------------------------------------------------------------------------
def _build_runs():
    """Per 512-tile: list of (lo, ln, i, k0) pieces.

    Gen-order column  B(i)+k  holds edge (i, i+1+k), k < padded width wp(i).
    """
    runs_by_tile = [[] for _ in range(NTILES)]
    pos = 0
    for i in range(NN - 1):
        w = NN - 1 - i
        wp = w + (w & 1)
        start = pos
        while start < pos + wp:
            t = start // TILE
            end = min(pos + wp, (t + 1) * TILE)
            runs_by_tile[t].append((start - t * TILE, end - start, i, start - pos))
            start = end
        pos += wp
    assert pos == CPAD
    return runs_by_tile


def _gather_index():
    """lex edge index -> gen-order position."""
    idx = np.empty(NE, np.int64)
    pos = 0
    e = 0
    for i in range(NN - 1):
        w = NN - 1 - i
        wp = w + (w & 1)
        idx[e:e + w] = np.arange(pos, pos + w)
        e += w
        pos += wp
    return idx


RUNS_BY_TILE = _build_runs()
GATHER = _gather_index()


def _chunk_runs(chunk):
    by_chunk = [[] for _ in range(CPAD // chunk)]
    pos = 0
    for i in range(NN - 1):
        w = NN - 1 - i
        wp = w + (w & 1)
        start = pos
        while start < pos + wp:
            c = start // chunk
            end = min(pos + wp, (c + 1) * chunk)
            by_chunk[c].append((start - c * chunk, end - start, i, start - pos))
            start = end
        pos += wp
    assert pos == CPAD
    return by_chunk

# Device feat row layout (32-aligned blocks for the engine partition-base rule):
#   rows  0:30  cos block (octave-major, xyz)
#   rows 30:32  zero pad
#   rows 32:62  sin block
#   rows 62:64  zero pad
#   rows 64:72  node_type rows
#   rows 72:76  quat rows
# W1 rows are permuted/padded on the host to match (pad rows = 0).
#
# PE features use exact fp32 range reduction: with h = 2^(i-1)*x (exact) and
# f = h - round(h) via the 2^23 magic constant, cos(2^i*pi*x) = cos(2*pi*f)
# = sin(pi/2 - 2*pi*|f|) and sin(2^i*pi*x) = sin(2*pi*f), all Sin args in
# [-pi, pi] (the ACT table's accurate domain).
DIN_PAD = 76
_COS = [8 + 6 * i + a for i in range(PE_OCT) for a in range(3)]
_SIN = [8 + 6 * i + 3 + a for i in range(PE_OCT) for a in range(3)]
MAGIC = float(2 ** 23)


def _prep_w1(W1):
    W1p = np.zeros((DIN_PAD, FD), np.float32)
    W1p[0:30] = W1[_COS]
    W1p[32:62] = W1[_SIN]
    W1p[64:72] = W1[0:8]
    W1p[72:76] = W1[68:72]
    return W1p


# --------------------------------------------------------------------------
# device kernel
# --------------------------------------------------------------------------
ABLATE = set()  # e.g. {"edgegen", "relu2", "wo", "mm2", "node"} — bench only
EDGE16 = True   # fp16 y/h1: DVE 4x mode edge-gen, fp16 mm2 with N=TILE
EDGE_ONEOP = False  # bench only: one full-tile edge-gen op per tile (wrong values)
ONLY_EDGEGEN = False  # bench only: drop mm2/relu2/wo — isolates DVE edge-gen
WC_ACT = 32   # anchor runs with padded width <= WC_ACT run on ScalarE
MMTILE = 1024  # mm2/relu2 tile width (psum pair-filled by two N=512 matmuls)
H1CHUNK = 4096  # h1 buffer chunk; runs split only at chunk boundaries


def build_kernel(repeat=1, loop_n=None):
    """repeat>1 re-emits the whole compute body N times; loop_n wraps the
    body in an on-device For_i loop — both only for timing in bench*.py."""
    nc = bacc.Bacc("TRN2", target_bir_lowering=False, debug=False)

    ntqT = nc.dram_tensor("ntqT", [TYPE_D + QUAT_D, BPC * NN], F32, kind="ExternalInput")
    posT = nc.dram_tensor("posT", [POS_D, BPC * NN], F32, kind="ExternalInput")
    w1 = nc.dram_tensor("w1", [DIN_PAD, FD], F32, kind="ExternalInput")
    w2 = nc.dram_tensor("w2", [FD, FD], F32, kind="ExternalInput")
    we1 = nc.dram_tensor("we1", [FD, FD], F32, kind="ExternalInput")
    we2 = nc.dram_tensor("we2", [FD, FD], F32, kind="ExternalInput")
    wo = nc.dram_tensor("wo", [FD, 1], F32, kind="ExternalInput")
    b1 = nc.dram_tensor("b1", [FD, 1], F32, kind="ExternalInput")
    b2 = nc.dram_tensor("b2", [FD, 1], F32, kind="ExternalInput")
    be1h = nc.dram_tensor("be1h", [FD, 1], F32, kind="ExternalInput")
    be2 = nc.dram_tensor("be2", [FD, 1], F32, kind="ExternalInput")
    octs = nc.dram_tensor("octs", [3 * PE_OCT, 1], F32, kind="ExternalInput")
    out = nc.dram_tensor("out", [BPC, FD, CPAD // FD], F32, kind="ExternalOutput")

    NCOL = BPC * NN  # 512

    with TileContext(nc) as tc:
        with (
            tc.tile_pool(name="const", bufs=1) as cpool,
            tc.tile_pool(name="h1p", bufs=4) as h1pool,
            tc.tile_pool(name="h2p", bufs=4) as h2pool,
            tc.tile_pool(name="sp", bufs=2) as spool,
            tc.tile_pool(name="psmm", bufs=3, space="PSUM") as psmm,
            tc.tile_pool(name="pss", bufs=2, space="PSUM") as pss,
        ):
            # ---- load constants -------------------------------------------------
            w1_sb = cpool.tile([DIN_PAD, FD], F32, tag="w1")
            w2_sb = cpool.tile([FD, FD], F32, tag="w2")
            we1_sb = cpool.tile([FD, FD], F32, tag="we1")
            we2_sb = cpool.tile([FD, FD], F32, tag="we2")
            wo32_sb = cpool.tile([FD, 1], F32, tag="wo32")
            wo_sb = cpool.tile([FD, 1], FP16, tag="wo")
            b1_sb = cpool.tile([FD, 1], F32, tag="b1")
            b2_sb = cpool.tile([FD, 1], F32, tag="b2")
            be1h_sb = cpool.tile([FD, 1], F32, tag="be1h")
            be2_sb = cpool.tile([FD, 1], F32, tag="be2")
            octs_sb = cpool.tile([3 * PE_OCT, 1], F32, tag="octs")
            for sb, dr in [
                (w1_sb, w1), (w2_sb, w2), (we1_sb, we1), (we2_sb, we2),
                (wo32_sb, wo), (b1_sb, b1), (b2_sb, b2), (be1h_sb, be1h),
                (be2_sb, be2), (octs_sb, octs),
            ]:
                nc.sync.dma_start(sb[:], dr[:])
            nc.vector.tensor_copy(wo_sb[:], wo32_sb[:])  # cast fp32 -> fp16
            we2_16 = cpool.tile([FD, FD], FP16, tag="we216")
            nc.vector.tensor_copy(we2_16[:], we2_sb[:])

            body_args = (nc, cpool, h1pool, h2pool, spool, psmm, pss,
                         w1_sb, w2_sb, we1_sb, we2_sb, we2_16, wo_sb,
                         b1_sb, b2_sb, be1h_sb, be2_sb, octs_sb,
                         ntqT, posT, out)
            if loop_n is not None:
                assert repeat == 1
                with tc.For_i(0, loop_n, 1):
                    _emit_body(*body_args)
            else:
                for _rep in range(repeat):
                    _emit_body(*body_args)

    nc.compile()
    return nc


def _emit_body(nc, cpool, h1pool, h2pool, spool, psmm, pss,
               w1_sb, w2_sb, we1_sb, we2_sb, we2_16, wo_sb,
               b1_sb, b2_sb, be1h_sb, be2_sb, octs_sb, ntqT, posT, out):
    NCOL = BPC * NN
    if True:
        if True:
            # ---- node stage (both batches share the 512-wide free dim) ----------
            feat = cpool.tile([FD, NCOL], F32, tag="feat")  # rows 0..71 used
            V = cpool.tile([3 * PE_OCT, NCOL], F32, tag="V")
            vt = cpool.tile([3 * PE_OCT, NCOL], F32, tag="vt")
            wt = cpool.tile([3 * PE_OCT, NCOL], F32, tag="wt")
            qt = cpool.tile([3 * PE_OCT, NCOL], F32, tag="qt")
            x1 = cpool.tile([FD, NCOL], F32, tag="x1")
            x2 = cpool.tile([FD, NCOL], F32, tag="x2")
            y = cpool.tile([FD, 2 * YSTRIDE], F32, tag="y")
            halfpi_sb = cpool.tile([3 * PE_OCT, 1], F32, tag="halfpi")
            nc.vector.memset(halfpi_sb[:], PI / 2)

            nc.vector.memset(feat[:], 0.0)
            nc.sync.dma_start(feat[64:76, :], ntqT[:])
            for k in range(PE_OCT):
                nc.sync.dma_start(V[3 * k:3 * k + 3, :], posT[:])
            # h = 2^(i-1)*x; r = round(h) via magic; f = h - r in [-0.5, 0.5]
            nc.vector.tensor_scalar(
                out=vt[:], in0=V[:], scalar1=octs_sb[:], scalar2=None, op0=ALU.mult)
            nc.vector.tensor_scalar(
                out=wt[:], in0=vt[:], scalar1=MAGIC, scalar2=MAGIC,
                op0=ALU.add, op1=ALU.subtract)
            nc.vector.tensor_sub(qt[:], vt[:], wt[:])          # f
            nc.scalar.activation(out=vt[:], in_=qt[:], func=ACTF.Abs,
                                 bias=0.0, scale=1.0)           # |f| (reuse vt)
            nc.scalar.activation(out=feat[0:30, :], in_=vt[:], func=ACTF.Sin,
                                 bias=halfpi_sb[:], scale=-2 * PI)
            nc.scalar.activation(out=feat[32:62, :], in_=qt[:], func=ACTF.Sin,
                                 bias=0.0, scale=2 * PI)

            ps1 = psmm.tile([FD, NCOL], F32, tag="mm")
            nc.tensor.matmul(out=ps1[:], lhsT=w1_sb[:], rhs=feat[0:DIN_PAD, :],
                             start=True, stop=True)
            nc.scalar.activation(out=x1[:], in_=ps1[:], func=ACTF.Relu, bias=b1_sb[:])
            ps2 = psmm.tile([FD, NCOL], F32, tag="mm")
            nc.tensor.matmul(out=ps2[:], lhsT=w2_sb[:], rhs=x1[:], start=True, stop=True)
            nc.scalar.activation(out=x2[:], in_=ps2[:], func=ACTF.Relu, bias=b2_sb[:])
            ps3 = psmm.tile([FD, NCOL], F32, tag="mm")
            nc.tensor.matmul(out=ps3[:], lhsT=we1_sb[:], rhs=x2[:], start=True, stop=True)
            nc.vector.memset(y[:], 0.0)
            for b in range(BPC):
                nc.scalar.activation(out=y[:, b * YSTRIDE:b * YSTRIDE + NN],
                                     in_=ps3[:, b * NN:(b + 1) * NN],
                                     func=ACTF.Identity, bias=be1h_sb[:])
            if EDGE16:
                # fp16 copies of y at both column parities so every edge-gen
                # read starts 4B-aligned (DVE 4x mode requirement)
                y16a = cpool.tile([FD, 2 * YSTRIDE], FP16, tag="y16a")
                y16b = cpool.tile([FD, 2 * YSTRIDE], FP16, tag="y16b")
                nc.vector.tensor_copy(y16a[:], y[:])
                nc.vector.tensor_copy(y16b[:, 0:2 * YSTRIDE - 1], y[:, 1:2 * YSTRIDE])
            else:
                y16a = y16b = None

            # ---- edge stage -----------------------------------------------------
            runs_by_chunk = _chunk_runs(H1CHUNK)
            TPC = H1CHUNK // MMTILE       # mm tiles per h1 chunk
            GT = 16 // (MMTILE // FD)     # mm tiles per ps_s group
            for b in range(BPC):
                yb = b * YSTRIDE
                s_sb = spool.tile([FD, CPAD // FD], F32, tag="s")
                if ONLY_EDGEGEN:
                    nc.vector.memset(s_sb[:], 0.0)
                for c in range(CPAD // H1CHUNK):
                    h1 = h1pool.tile([FD, H1CHUNK], FP16 if EDGE16 else F32, tag="h1")
                    if "edgegen" not in ABLATE:
                        for (lo, ln, i, k0) in runs_by_chunk[c]:
                            c0 = yb + i + 1 + k0
                            if ln <= WC_ACT:
                                nc.scalar.activation(
                                    out=h1[:, lo:lo + ln],
                                    in_=y[:, c0:c0 + ln],
                                    func=ACTF.Relu,
                                    bias=y[:, yb + i:yb + i + 1], scale=1.0)
                            else:
                                if EDGE16:
                                    src = y16a[:, c0:c0 + ln] if c0 % 2 == 0 \
                                        else y16b[:, c0 - 1:c0 - 1 + ln]
                                else:
                                    src = y[:, c0:c0 + ln]
                                nc.vector.tensor_scalar(
                                    out=h1[:, lo:lo + ln],
                                    in0=src,
                                    scalar1=y[:, yb + i: yb + i + 1],
                                    scalar2=0.0, op0=ALU.add, op1=ALU.max)
                    if ONLY_EDGEGEN:
                        continue
                    for tt in range(TPC):
                        t = TPC * c + tt          # global mm-tile index
                        if t % GT == 0:
                            ps_s = pss.tile([FD, 16], F32, tag="ps_s")
                        ps_e = psmm.tile([FD, MMTILE], F32, tag="mm")
                        if "mm2" not in ABLATE:
                            for hh in range(MMTILE // 512):
                                nc.tensor.matmul(
                                    out=ps_e[:, 512 * hh:512 * (hh + 1)],
                                    lhsT=we2_16[:] if EDGE16 else we2_sb[:],
                                    rhs=h1[:, MMTILE * tt + 512 * hh:
                                            MMTILE * tt + 512 * (hh + 1)],
                                    start=True, stop=True)
                        h2 = h2pool.tile([FD, MMTILE], FP16, tag="h2")
                        if "relu2" not in ABLATE:
                            nc.scalar.activation(out=h2[:], in_=ps_e[:], func=ACTF.Relu,
                                                 bias=be2_sb[:])
                        if "wo" not in ABLATE:
                            for k in range(MMTILE // FD):
                                nc.tensor.matmul(
                                    out=ps_s[:, (MMTILE // FD) * (t % GT) + k:
                                             (MMTILE // FD) * (t % GT) + k + 1],
                                    lhsT=h2[:, FD * k:FD * (k + 1)],
                                    rhs=wo_sb[:], start=True, stop=True)
                        if t % GT == GT - 1:
                            nc.vector.tensor_copy(
                                out=s_sb[:, 16 * (t // GT):16 * (t // GT + 1)],
                                in_=ps_s[:])
                nc.sync.dma_start(out[b], s_sb[:])


_NC_CACHE = None


def _get_nc():
    global _NC_CACHE
    if _NC_CACHE is None:
        _NC_CACHE = build_kernel()
    return _NC_CACHE


# --------------------------------------------------------------------------
# host wrapper
# --------------------------------------------------------------------------
def kernel(node_type, pos, quat, W1, b1, W2, b2, We1, be1, We2, be2, Wo, bo):
    node_type = np.asarray(node_type, np.float32)
    pos = np.asarray(pos, np.float32)
    quat = np.asarray(quat, np.float32)

    W1p = _prep_w1(np.asarray(W1, np.float32))

    octscale = np.repeat((2.0 ** (np.arange(PE_OCT) - 1)).astype(np.float32), 3)[:, None]
    col = lambda v: np.ascontiguousarray(np.asarray(v, np.float32).reshape(FD, 1))

    shared = {
        "w1": np.ascontiguousarray(W1p),
        "w2": np.ascontiguousarray(np.asarray(W2, np.float32)),
        "we1": np.ascontiguousarray(np.asarray(We1, np.float32)),
        "we2": np.ascontiguousarray(np.asarray(We2, np.float32)),
        "wo": np.ascontiguousarray(np.asarray(Wo, np.float32).reshape(FD, 1)),
        "b1": col(b1), "b2": col(b2),
        "be1h": col(np.asarray(be1, np.float32) * 0.5), "be2": col(be2),
        "octs": np.ascontiguousarray(octscale),
    }

    in_maps = []
    for c in range(NCORES):
        bs = slice(c * BPC, (c + 1) * BPC)
        nt = node_type[bs]          # [2, 256, 8]
        qt = quat[bs]               # [2, 256, 4]
        ps = pos[bs]                # [2, 256, 3]
        ntq = np.concatenate(
            [nt.transpose(2, 0, 1).reshape(TYPE_D, -1),
             qt.transpose(2, 0, 1).reshape(QUAT_D, -1)], axis=0)
        in_maps.append({
            "ntqT": np.ascontiguousarray(ntq),
            "posT": np.ascontiguousarray(ps.transpose(2, 0, 1).reshape(POS_D, -1)),
            **shared,
        })

    nc = _get_nc()
    results = run_bass_kernel_spmd(nc, in_maps, core_ids=list(range(NCORES))).results

    out = np.empty((NB, NE), np.float32)
    bo0 = np.float32(np.asarray(bo).reshape(-1)[0])
    for c in range(NCORES):
        r = results[c]["out"]  # [2, 128, 256]; element (p, col) = gen idx col*128+p
        for b in range(BPC):
            gen = np.ascontiguousarray(r[b].T).reshape(-1)
            out[c * BPC + b] = gen[GATHER] + bo0
    return out
